# revision 44
# baseline (speedup 1.0000x reference)
"""Trainium2 Bass kernel for nn_DistanceDecoder (GCN stack + per-edge MLPs).

Strategy (8 NeuronCores, SPMD):
  - Nodes are permuted and sharded across cores (stratified by degree so every
    128-node block has a near-equal number of incoming edges).
  - Edges (incl. self loops) are bucketed by destination block; segment-sum is
    done as a one-hot matmul accumulating in PSUM (lhsT = one-hot[edge, node],
    rhs = gathered source features).
  - norm = dinv[s]*dinv[d] is folded into a pre-scale of the feature table by
    dinv (before AllGather) and a post-scale of the block output by dinv.
  - Per layer: transform own shard -> AllGather bf16 table -> dma_gather source
    rows -> one-hot matmul segment sum -> bias/relu.
  - Edge stage: transpose-mode dma_gather of z rows (raw, AllGathered at the
    start so it overlaps the GCN phase) and g rows for src and dst; MLP hidden
    layer, r/t contractions and dist^2 column sums all as PE matmuls in the
    feature-on-partition orientation; leaky-relu on ACT (Lrelu) directly from
    PSUM; final sigmoid on ACT.
  - dma_gather indices are int16, so every table is split in lo/hi halves and
    each chunk of 128 edges is (compile-time) assigned to one half.

Harness contract: kernel(**inputs) takes full inputs, returns full [E] f32.
"""

import math
import numpy as np

P = 128
NCORES = 8
ZD = 128
HD = 256
HD2 = HD // 2


# --------------------------------------------------------------------------
# Host-side planning (integer work only: permutation, bucketing, padding)
# --------------------------------------------------------------------------

def build_plan(edge_index, N, ncores=NCORES, gb=None, eb=32):
    src = edge_index[0].astype(np.int64)
    dst = edge_index[1].astype(np.int64)
    E = src.shape[0]

    npc = int(math.ceil(N / ncores / P)) * P      # nodes per core (padded)
    if (npc // P) % 2:
        npc += P                                   # even block count -> gb=2
    npad = npc * ncores
    half = npad // 2
    nb = npc // P                                  # blocks per core
    nblk = npad // P                               # global blocks
    assert half <= 32768, "int16 gather index range exceeded"

    deg = np.bincount(dst, minlength=N).astype(np.int64) + 1   # + self loop

    # stratified permutation: sort by degree desc, deal round-robin over all
    # global blocks; global block i -> (core i % ncores, local block i//ncores)
    order = np.argsort(-deg, kind="stable")
    i = np.arange(N)
    gblk = i % nblk
    slot = i // nblk
    core = gblk % ncores
    lblk = gblk // ncores
    pid = core * npc + lblk * P + slot
    old2new = np.empty(N, np.int64)
    old2new[order] = pid
    new2old = np.full(npad, -1, np.int64)
    new2old[pid] = order

    deg_pad = np.ones(npad, np.float32)
    deg_pad[old2new] = deg.astype(np.float32)

    # ---- scaled-edge lists (E edges + N self loops), bucketed by dst block
    s_all = old2new[np.concatenate([src, np.arange(N)])]
    d_all = old2new[np.concatenate([dst, np.arange(N)])]
    ecore = d_all // npc
    eblk = (d_all % npc) // P
    ecol = d_all % P
    ehalf = (s_all >= half).astype(np.int64)
    srel = s_all - ehalf * half

    # group edges by (core, block, half)
    key = (ecore * nb + eblk) * 2 + ehalf
    ordk = np.argsort(key, kind="stable")
    key_s = key[ordk]
    srel_s = srel[ordk]
    ecol_s = ecol[ordk]
    nkeys = ncores * nb * 2
    counts = np.bincount(key_s, minlength=nkeys)
    # per-block chunk counts (maxed over cores so the SPMD program is shared)
    cview = counts.reshape(ncores, nb, 2)
    cl_b = tuple(int(math.ceil(cview[:, b, 0].max() / P)) for b in range(nb))
    ch_b = tuple(int(math.ceil(cview[:, b, 1].max() / P)) for b in range(nb))
    cl = sum(cl_b)                  # total lo chunks per core
    ch = sum(ch_b)                  # total hi chunks per core
    off_l = np.concatenate([[0], np.cumsum(cl_b)]).astype(int)
    off_h = np.concatenate([[0], np.cumsum(ch_b)]).astype(int)

    # packed per-(core,block,half) chunk arrays (flat, variable per block)
    dinv_pad = deg_pad.astype(np.float64) ** -0.5
    dval_s = dinv_pad[s_all[ordk]].astype(np.float32)  # dinv of source/slot
    gidx_lo = np.zeros((ncores, cl * P), np.int16)
    gidx_hi = np.zeros((ncores, ch * P), np.int16)
    dcol_lo = np.full((ncores, cl * P), -1.0, np.float32)
    dcol_hi = np.full((ncores, ch * P), -1.0, np.float32)
    dval_lo = np.zeros((ncores, cl * P), np.float32)
    dval_hi = np.zeros((ncores, ch * P), np.float32)
    starts = np.concatenate([[0], np.cumsum(counts)])
    for c in range(ncores):
        for b in range(nb):
            for h in range(2):
                k = (c * nb + b) * 2 + h
                n = cview[c, b, h]
                sl = slice(starts[k], starts[k] + n)
                if h == 0:
                    base = off_l[b] * P
                    gidx_lo[c, base:base + n] = srel_s[sl]
                    dcol_lo[c, base:base + n] = ecol_s[sl]
                    dval_lo[c, base:base + n] = dval_s[sl]
                else:
                    base = off_h[b] * P
                    gidx_hi[c, base:base + n] = srel_s[sl]
                    dcol_hi[c, base:base + n] = ecol_s[sl]
                    dval_hi[c, base:base + n] = dval_s[sl]

    # wrap indices into the [16, n/16] layout dma_gather expects
    def wrap16(a):   # [..., M] int16 -> [128, total/16]: wrapped in 16
        # partitions, replicated into each of the 8 16-partition groups
        # (each GPSIMD Q7 core pair reads its own group).
        flat = a.reshape(-1, 16)
        w = np.ascontiguousarray(flat.T).astype(np.int16)
        return np.ascontiguousarray(np.tile(w, (8, 1)))

    # dstcol in device layout [P, nchunks] (partition-major)
    def colmajor(a):  # [ncores, C*P] -> per core [P, C]
        out = []
        for c in range(ncores):
            m = a[c].reshape(-1, P)   # [C, P]
            out.append(np.ascontiguousarray(m.T).astype(np.float32))
        return out

    # ---- edge stage: original E edges, round robin over cores, 4 combos
    es = old2new[src]
    ed = old2new[dst]
    ecore2 = np.arange(E) % ncores
    combo = (es >= half).astype(np.int64) * 2 + (ed >= half).astype(np.int64)
    key2 = ecore2 * 4 + combo
    ordk2 = np.argsort(key2, kind="stable")
    counts2 = np.bincount(key2[ordk2], minlength=ncores * 4).reshape(ncores, 4)
    ecs = [max(1, int(math.ceil(counts2[:, k].max() / P))) for k in range(4)]
    nck = sum(ecs)

    eidx_src = np.zeros((ncores, nck * P), np.int16)
    eidx_dst = np.zeros((ncores, nck * P), np.int16)
    slotmap = np.full((ncores, nck * P), -1, np.int64)
    starts2 = np.concatenate([[0], np.cumsum(counts2.reshape(-1))])
    es_rel = (es - (es >= half) * half).astype(np.int16)
    ed_rel = (ed - (ed >= half) * half).astype(np.int16)
    for c in range(ncores):
        off = 0
        for k in range(4):
            kk = c * 4 + k
            n = counts2[c, k]
            sl = ordk2[starts2[kk]:starts2[kk] + n]
            eidx_src[c, off:off + n] = es_rel[sl]
            eidx_dst[c, off:off + n] = ed_rel[sl]
            slotmap[c, off:off + n] = sl
            off += ecs[k] * P

    if gb is None:
        gb = 1
        for g in (2, 5):
            if nb % g == 0:
                gb = g
                break

    meta = dict(npc=npc, npad=npad, half=half, nb=nb, cl=cl, ch=ch,
                cl_b=cl_b, ch_b=ch_b,
                ecs=tuple(ecs), nck=nck, gb=gb, eb=eb, ncores=ncores)
    percore = []
    dcl = colmajor(dcol_lo)
    dch = colmajor(dcol_hi)
    dvl = colmajor(dval_lo)
    dvh = colmajor(dval_hi)
    for c in range(ncores):
        percore.append(dict(
            gidx_lo=wrap16(gidx_lo[c]),
            gidx_hi=wrap16(gidx_hi[c]),
            dcol_lo=dcl[c],
            dcol_hi=dch[c],
            dval_lo=dvl[c],
            dval_hi=dvh[c],
            eidx_src=wrap16(eidx_src[c]),
            eidx_dst=wrap16(eidx_dst[c]),
        ))
    host = dict(old2new=old2new, new2old=new2old, deg_pad=deg_pad,
                slotmap=slotmap)
    return meta, percore, host


# --------------------------------------------------------------------------
# Bass program
# --------------------------------------------------------------------------

def build_nc(meta, debug=False):
    import concourse.bacc as bacc
    import concourse.tile as tile
    from concourse import mybir

    f32 = mybir.dt.float32
    bf16 = mybir.dt.bfloat16
    i16 = mybir.dt.int16
    AF = mybir.ActivationFunctionType
    OP = mybir.AluOpType

    npc, npad, half = meta["npc"], meta["npad"], meta["half"]
    nb, cl, ch = meta["nb"], meta["cl"], meta["ch"]
    cl_b, ch_b = meta["cl_b"], meta["ch_b"]
    off_l = [0]
    for c_ in cl_b:
        off_l.append(off_l[-1] + c_)
    off_h = [0]
    for c_ in ch_b:
        off_h.append(off_h[-1] + c_)
    ecs, nck = meta["ecs"], meta["nck"]
    gb, eb = meta["gb"], meta["eb"]
    ncores = meta["ncores"]
    # compile-time specialization: biases that are identically zero in the
    # staged inputs are elided from the program.
    zb_gcn, zb_edge, zb_br2 = meta.get("zbias", (False, False, False))
    rg = [list(range(ncores))]

    nc = bacc.Bacc("TRN2", target_bir_lowering=False, debug=debug,
                   num_devices=ncores)

    def din(name, shape, dtype):
        return nc.dram_tensor(name, list(shape), dtype, kind="ExternalInput")

    z_d = din("z_shard", [npc, ZD], f32)
    degt_d = din("deg_t", [P, nb], f32)
    glo_d = din("gidx_lo", [P, cl * 8], i16)
    ghi_d = din("gidx_hi", [P, ch * 8], i16)
    dcl_d = din("dcol_lo", [P, cl], f32)
    dch_d = din("dcol_hi", [P, ch], f32)
    dvl_d = din("dval_lo", [P, cl], f32)
    dvh_d = din("dval_hi", [P, ch], f32)
    esrc_d = din("eidx_src", [P, nck * 8], i16)
    edst_d = din("eidx_dst", [P, nck * 8], i16)
    W0_d = din("W0", [ZD, HD], f32)
    W1_d = din("W1", [HD, HD], f32)
    W2_d = din("W2", [HD, HD], f32)
    W3_d = din("W3", [HD, HD2], f32)
    b0_d = din("b0c", [P, HD], f32)
    b1_d = din("b1c", [P, HD], f32)
    b2_d = din("b2c", [P, HD], f32)
    b3_d = din("b3c", [P, HD2], f32)
    wsrc_d = din("wsrc_cat", [HD2, 2 * HD], f32)
    wdst_d = din("wdst_cat", [HD2, 2 * HD], f32)
    w2q_d = din("w2q", [P, 8], f32)             # [128, 4 quarters, 2 (r|t)]
    brt_d = din("brt_cat", [1, 2 * HD], f32)    # [br1 | bt1]
    br2_d = din("br2bt2", [P, 2], f32)          # col0 br2, col1 bt2
    iota_d = din("iota_f", [P, P], f32)
    identf_d = din("ident_f", [P, P], f32)

    out_d = nc.dram_tensor("out", [P, nck], f32, kind="ExternalOutput")

    from concourse import library_config
    with tile.TileContext(nc) as tc:
        nc.gpsimd.load_library(library_config.mlp)
        with tc.tile_pool(name="dram", bufs=1, space="DRAM") as dram, \
             tc.tile_pool(name="cpool", bufs=1) as cpool, \
             tc.tile_pool(name="spool", bufs=3) as spool, \
             tc.tile_pool(name="dpool", bufs=2) as dpool, \
             tc.tile_pool(name="dps", bufs=2, space="PSUM") as dps:

            # ---------- DRAM intermediates ----------
            zp_shard = dram.tile([npc, ZD], bf16)       # raw z (bf16)
            zp_full = dram.tile([npad, ZD], bf16, addr_space="Shared")
            u1_shard = dram.tile([npc, HD], bf16)
            u1_full = dram.tile([npad, HD], bf16, addr_space="Shared")
            u2_shard = dram.tile([npc, HD], bf16)
            u2_full = dram.tile([npad, HD], bf16, addr_space="Shared")
            t3_shard = dram.tile([npc, HD2], bf16)
            t3_full = dram.tile([npad, HD2], bf16, addr_space="Shared")
            g_shard = dram.tile([npc, HD2], bf16)
            g_full = dram.tile([npad, HD2], bf16, addr_space="Shared")

            # ---------- constants into SBUF ----------
            def load_const(dap, shape, dtype, name):
                t = cpool.tile(list(shape), dtype, name=name)
                nc.sync.dma_start(out=t[:], in_=dap)
                return t

            def load_const_bf(dap, shape, name):
                tf = spool.tile(list(shape), f32, name=name + "_f", tag="cvt")
                nc.sync.dma_start(out=tf[:], in_=dap)
                tb = cpool.tile(list(shape), bf16, name=name)
                nc.scalar.copy(out=tb[:], in_=tf[:])
                return tb

            iota_sb = load_const(iota_d.ap(), [P, P], f32, "iota_sb")
            identf_sb = load_const(identf_d.ap(), [P, P], f32, "identf_sb")
            identb_sb = cpool.tile([P, P], bf16, name="identb_sb")
            nc.vector.tensor_copy(out=identb_sb[:], in_=identf_sb[:])
            b0_sb = load_const(b0_d.ap(), [P, HD], f32, "b0_sb")
            b1_sb = load_const(b1_d.ap(), [P, HD], f32, "b1_sb")
            b2_sb = load_const(b2_d.ap(), [P, HD], f32, "b2_sb")
            b3_sb = load_const(b3_d.ap(), [P, HD2], f32, "b3_sb")
            W0_sb = load_const_bf(W0_d.ap(), [ZD, HD], "W0_sb")
            W1a_sb = load_const_bf(W1_d.ap()[0:P, :], [P, HD], "W1a_sb")
            W1b_sb = load_const_bf(W1_d.ap()[P:HD, :], [P, HD], "W1b_sb")
            W2a_sb = load_const_bf(W2_d.ap()[0:P, :], [P, HD], "W2a_sb")
            W2b_sb = load_const_bf(W2_d.ap()[P:HD, :], [P, HD], "W2b_sb")
            W3a_sb = load_const_bf(W3_d.ap()[0:P, :], [P, HD2], "W3a_sb")
            W3b_sb = load_const_bf(W3_d.ap()[P:HD, :], [P, HD2], "W3b_sb")
            wsrc_sb = load_const_bf(wsrc_d.ap(), [HD2, 2 * HD], "wsrc_sb")
            wdst_sb = load_const_bf(wdst_d.ap(), [HD2, 2 * HD], "wdst_sb")
            w2q_sb = load_const_bf(w2q_d.ap(), [P, 8], "w2q_sb")
            brt_sb = load_const_bf(brt_d.ap(), [1, 2 * HD], "brt_sb")
            br2_sb = load_const(br2_d.ap(), [P, 2], f32, "br2_sb")
            ones2_sb = cpool.tile([1, 2 * P], bf16, name="ones2_sb")
            nc.vector.memset(ones2_sb[:], 1.0)
            onesc_sb = cpool.tile([P, 1], bf16, name="onesc_sb")
            nc.vector.memset(onesc_sb[:], 1.0)

            dcl_sb = load_const(dcl_d.ap(), [P, cl], f32, "dcl_sb")
            dch_sb = load_const(dch_d.ap(), [P, ch], f32, "dch_sb")
            dvl_sb = load_const(dvl_d.ap(), [P, cl], f32, "dvl_sb")
            dvh_sb = load_const(dvh_d.ap(), [P, ch], f32, "dvh_sb")
            glo_sb = load_const(glo_d.ap(), [P, cl * 8], i16, "glo_sb")
            ghi_sb = load_const(ghi_d.ap(), [P, ch * 8], i16, "ghi_sb")
            esrc_sb = load_const(esrc_d.ap(), [P, nck * 8], i16, "esrc_sb")
            edst_sb = load_const(edst_d.ap(), [P, nck * 8], i16, "edst_sb")

            # dinv = sqrt(1/deg)
            deg_sb = load_const(degt_d.ap(), [P, nb], f32, "deg_sb")
            rec_sb = cpool.tile([P, nb], f32, name="rec_sb")
            nc.vector.reciprocal(out=rec_sb[:], in_=deg_sb[:])
            dinv_sb = cpool.tile([P, nb], f32, name="dinv_sb")
            nc.scalar.sqrt(out=dinv_sb[:], in_=rec_sb[:])

            # ---------- dist pipeline (interleaved into AllGather windows) --
            # nrm_sb[:, slot] = ||z_src - z_dst|| per edge slot. Depends only
            # on the raw-z table (AllGathered first), so batches of it are
            # processed inside the GCN-phase collective windows where every
            # engine would otherwise idle.
            nrm_sb = cpool.tile([P, nck], f32, name="nrm_sb")
            combo_base = [0]
            for k in range(3):
                combo_base.append(combo_base[-1] + ecs[k])
            dist_state = {"k": 0, "c0": 0}

            def dist_batches(nbatches):
                zlo_t = zp_full[0:half, :]
                zhi_t = zp_full[half:npad, :]
                done = 0
                while done < nbatches and dist_state["k"] < 4:
                    k, c0 = dist_state["k"], dist_state["c0"]
                    nchunks = ecs[k]
                    nbch = min(eb, nchunks - c0)
                    base = combo_base[k] + c0
                    s_z = zhi_t if k >= 2 else zlo_t
                    d_z = zhi_t if (k % 2) else zlo_t
                    zs = dpool.tile([P, 1, eb * P], bf16, name="zs", tag="zs")
                    nc.gpsimd.dma_gather(
                        out_ap=zs[:, :, 0:nbch * P], in_ap=s_z,
                        idxs_ap=esrc_sb[:, base * 8:(base + nbch) * 8],
                        num_idxs=nbch * P, num_idxs_reg=nbch * P,
                        elem_size=ZD, transpose=True, single_packet=False)
                    zd = dpool.tile([P, 1, eb * P], bf16, name="zd", tag="zd")
                    nc.gpsimd.dma_gather(
                        out_ap=zd[:, :, 0:nbch * P], in_ap=d_z,
                        idxs_ap=edst_sb[:, base * 8:(base + nbch) * 8],
                        num_idxs=nbch * P, num_idxs_reg=nbch * P,
                        elem_size=ZD, transpose=True, single_packet=False)
                    ps_d = dps.tile([P, eb], f32, name="ps_d", tag="ps_d")
                    for cc0 in range(0, nbch, 2):
                        w = min(2, nbch - cc0)
                        cols = w * P
                        diffp = dpool.tile([P, 2 * P], bf16, name="diffp",
                                           tag="diffp")
                        nc.vector.tensor_tensor(
                            out=diffp[:, 0:cols],
                            in0=zs[:, 0, cc0 * P:cc0 * P + cols],
                            in1=zd[:, 0, cc0 * P:cc0 * P + cols],
                            op=OP.subtract)
                        sqp = dpool.tile([P, 2 * P], bf16, name="sqp",
                                         tag="sqp")
                        nc.vector.tensor_tensor(
                            out=sqp[:, 0:cols], in0=diffp[:, 0:cols],
                            in1=diffp[:, 0:cols], op=OP.mult)
                        for i in range(w):
                            cc = cc0 + i
                            nc.tensor.matmul(
                                ps_d[:, cc:cc + 1],
                                lhsT=sqp[:, i * P:(i + 1) * P],
                                rhs=onesc_sb[:], start=True, stop=True)
                    nc.scalar.sqrt(out=nrm_sb[:, base:base + nbch],
                                   in_=ps_d[:, 0:nbch])
                    done += 1
                    c0 += eb
                    if c0 >= nchunks:
                        dist_state["k"] = k + 1
                        dist_state["c0"] = 0
                    else:
                        dist_state["c0"] = c0

            # ---------- GCN phase ----------
            with tc.tile_pool(name="hpool", bufs=2) as hpool, \
                 tc.tile_pool(name="gpool", bufs=2) as gpool, \
                 tc.tile_pool(name="ohpool", bufs=4) as ohpool, \
                 tc.tile_pool(name="psum", bufs=2, space="PSUM") as psum:

                # phase B: raw-z table, staged 4 blocks per DMA. The GCN
                # norm's source-side dinv is folded into the one-hot values,
                # so the table itself is unscaled (and doubles as the dist
                # table).
                zbw = 4
                for b0 in range(0, nb, zbw):
                    w = min(zbw, nb - b0)
                    zb = spool.tile([P, zbw, ZD], f32, name="zb", tag="zb")
                    src = z_d.ap()[b0 * P:(b0 + w) * P, :]
                    nc.sync.dma_start(
                        out=zb[:, 0:w, :],
                        in_=src.rearrange("(b p) d -> p b d", p=P))
                    zpb = spool.tile([P, zbw, ZD], bf16, name="zpb",
                                     tag="zpb")
                    nc.scalar.copy(out=zpb[:, 0:w, :], in_=zb[:, 0:w, :])
                    dst = zp_shard[b0 * P:(b0 + w) * P, :]
                    nc.sync.dma_start(
                        out=dst.rearrange("(b p) d -> p b d", p=P),
                        in_=zpb[:, 0:w, :])
                nc.gpsimd.collective_compute(
                    "AllGather", OP.bypass, replica_groups=rg,
                    ins=[zp_shard[:].opt()], outs=[zp_full[:].opt()])

                h1_sb = hpool.tile([P, nb, HD], bf16, name="h1_sb", tag="h")
                h2_sb = hpool.tile([P, nb, HD], bf16, name="h2_sb", tag="h")
                h3_sb = hpool.tile([P, nb, HD], bf16, name="h3_sb", tag="h")

                gclmax = max(off_l[b0 + gb] - off_l[b0]
                             for b0 in range(0, nb, gb))
                gchmax = max(off_h[b0 + gb] - off_h[b0]
                             for b0 in range(0, nb, gb))

                def propagate(table, width, epilogue):
                    tlo = table[0:half, :]
                    thi = table[half:npad, :]
                    for grp in range(nb // gb):
                        b0 = grp * gb
                        gcl = off_l[b0 + gb] - off_l[b0]
                        gch = off_h[b0 + gb] - off_h[b0]
                        glo = gpool.tile([P, gclmax, width], bf16,
                                         name="glo", tag="glo")
                        nc.gpsimd.dma_gather(
                            out_ap=glo[:, 0:gcl, :], in_ap=tlo,
                            idxs_ap=glo_sb[:, off_l[b0] * 8:
                                           (off_l[b0] + gcl) * 8],
                            num_idxs=gcl * P, num_idxs_reg=gcl * P,
                            elem_size=width, single_packet=False)
                        ghi_t = gpool.tile([P, gchmax, width], bf16,
                                           name="ghi_t", tag="ghi")
                        nc.gpsimd.dma_gather(
                            out_ap=ghi_t[:, 0:gch, :], in_ap=thi,
                            idxs_ap=ghi_sb[:, off_h[b0] * 8:
                                           (off_h[b0] + gch) * 8],
                            num_idxs=gch * P, num_idxs_reg=gch * P,
                            elem_size=width, single_packet=False)
                        for bb in range(gb):
                            b = b0 + bb
                            ncl, nch = cl_b[b], ch_b[b]
                            total = ncl + nch
                            ps = psum.tile([P, width], f32, name="prop_ps",
                                           tag="prop")
                            idx = 0
                            for j in range(ncl):
                                col = off_l[b] + j
                                oh = ohpool.tile([P, P], bf16, name="oh",
                                                 tag="oh")
                                nc.vector.tensor_scalar(
                                    out=oh[:], in0=iota_sb[:],
                                    scalar1=dcl_sb[:, col:col + 1],
                                    scalar2=dvl_sb[:, col:col + 1],
                                    op0=OP.is_equal, op1=OP.mult)
                                nc.tensor.matmul(
                                    ps[:], lhsT=oh[:],
                                    rhs=glo[:, off_l[b] - off_l[b0] + j, :],
                                    start=(idx == 0),
                                    stop=(idx == total - 1))
                                idx += 1
                            for j in range(nch):
                                col = off_h[b] + j
                                oh = ohpool.tile([P, P], bf16, name="oh",
                                                 tag="oh")
                                nc.vector.tensor_scalar(
                                    out=oh[:], in0=iota_sb[:],
                                    scalar1=dch_sb[:, col:col + 1],
                                    scalar2=dvh_sb[:, col:col + 1],
                                    op0=OP.is_equal, op1=OP.mult)
                                nc.tensor.matmul(
                                    ps[:], lhsT=oh[:],
                                    rhs=ghi_t[:, off_h[b] - off_h[b0] + j, :],
                                    start=(idx == 0),
                                    stop=(idx == total - 1))
                                idx += 1
                            epilogue(b, ps)

                def epi0(b, ps):
                    s0 = spool.tile([P, ZD], bf16, name="s0", tag="s0")
                    nc.scalar.mul(out=s0[:], in_=ps[:],
                                  mul=dinv_sb[:, b:b + 1])
                    s0t_ps = psum.tile([P, P], bf16, name="s0t_ps", tag="tp")
                    nc.tensor.transpose(s0t_ps[:], s0[:], identb_sb[:])
                    s0t = spool.tile([P, P], bf16, name="s0t", tag="s0t")
                    nc.vector.tensor_copy(out=s0t[:], in_=s0t_ps[:])
                    hps = psum.tile([P, HD], f32, name="hps", tag="mm")
                    nc.tensor.matmul(hps[:], lhsT=s0t[:], rhs=W0_sb[:],
                                     start=True, stop=True)
                    if not zb_gcn:
                        nc.vector.tensor_tensor(out=hps[:], in0=hps[:],
                                                in1=b0_sb[:], op=OP.add)
                    nc.scalar.activation(h1_sb[:, b, :], hps[:], AF.Relu)

                propagate(zp_full, ZD, epi0)

                def transform(h_sb, wts, outw, dest):
                    for b in range(nb):
                        ups = psum.tile([P, outw], f32, name="ups", tag="mm")
                        nkh = len(wts)
                        for kh in range(nkh):
                            ht_ps = psum.tile([P, P], bf16, name="ht_ps",
                                              tag="tp")
                            nc.tensor.transpose(
                                ht_ps[:], h_sb[:, b, kh * P:(kh + 1) * P],
                                identb_sb[:])
                            ht = spool.tile([P, P], bf16, name="ht", tag="ht")
                            nc.vector.tensor_copy(out=ht[:], in_=ht_ps[:])
                            nc.tensor.matmul(ups[:], lhsT=ht[:],
                                             rhs=wts[kh][:],
                                             start=(kh == 0),
                                             stop=(kh == nkh - 1))
                        usb = spool.tile([P, outw], bf16, name="usb",
                                         tag="usb")
                        nc.scalar.copy(out=usb[:], in_=ups[:])
                        nc.sync.dma_start(out=dest[b * P:(b + 1) * P, :],
                                          in_=usb[:])

                def epi_mid(h_next, bias_sb):
                    def epi(b, ps):
                        nc.vector.tensor_scalar_mul(ps[:], ps[:],
                                                    dinv_sb[:, b:b + 1])
                        if not zb_gcn:
                            nc.vector.tensor_tensor(out=ps[:], in0=ps[:],
                                                    in1=bias_sb[:],
                                                    op=OP.add)
                        nc.scalar.activation(h_next[:, b, :], ps[:], AF.Relu)
                    return epi

                transform(h1_sb, [W1a_sb, W1b_sb], HD, u1_shard)
                nc.gpsimd.collective_compute(
                    "AllGather", OP.bypass, replica_groups=rg,
                    ins=[u1_shard[:].opt()], outs=[u1_full[:].opt()])
                dist_batches(7)
                propagate(u1_full, HD, epi_mid(h2_sb, b1_sb))

                transform(h2_sb, [W2a_sb, W2b_sb], HD, u2_shard)
                nc.gpsimd.collective_compute(
                    "AllGather", OP.bypass, replica_groups=rg,
                    ins=[u2_shard[:].opt()], outs=[u2_full[:].opt()])
                dist_batches(7)
                propagate(u2_full, HD, epi_mid(h3_sb, b2_sb))

                transform(h3_sb, [W3a_sb, W3b_sb], HD2, t3_shard)
                nc.gpsimd.collective_compute(
                    "AllGather", OP.bypass, replica_groups=rg,
                    ins=[t3_shard[:].opt()], outs=[t3_full[:].opt()])
                dist_batches(5)

                def epi3(b, ps):
                    nc.vector.tensor_scalar_mul(ps[:], ps[:],
                                                dinv_sb[:, b:b + 1])
                    if not zb_gcn:
                        nc.vector.tensor_tensor(out=ps[:], in0=ps[:],
                                                in1=b3_sb[:], op=OP.add)
                    gb_ = spool.tile([P, HD2], bf16, name="gb_", tag="gb_")
                    nc.scalar.copy(out=gb_[:], in_=ps[:])
                    nc.sync.dma_start(out=g_shard[b * P:(b + 1) * P, :],
                                      in_=gb_[:])

                propagate(t3_full, HD2, epi3)
                nc.gpsimd.collective_compute(
                    "AllGather", OP.bypass, replica_groups=rg,
                    ins=[g_shard[:].opt()], outs=[g_full[:].opt()])
                dist_batches(100)   # flush the remainder into the g window

            # ---------- edge stage ----------
            # Transposed orientation: gathered features sit on partitions
            # (lhsT), so the MLP hidden layer, the r/t contractions and the
            # dist^2 column sums are all PE matmuls; the only DVE work per
            # chunk is diff & square. Leaky-relu runs on ACT straight out of
            # PSUM (Lrelu, alpha=0.2).
            with tc.tile_pool(name="epool", bufs=2) as epool, \
                 tc.tile_pool(name="fpool", bufs=3) as fpool, \
                 tc.tile_pool(name="jpool", bufs=4) as jpool, \
                 tc.tile_pool(name="hps_pool", bufs=2, space="PSUM") as hps_pool, \
                 tc.tile_pool(name="rt_pool", bufs=2, space="PSUM") as rt_pool:

                glo_t = g_full[0:half, :]
                ghi_t = g_full[half:npad, :]

                for k in range(4):
                    s_g = ghi_t if k >= 2 else glo_t
                    d_g = ghi_t if (k % 2) else glo_t
                    nchunks = ecs[k]
                    for c0 in range(0, nchunks, eb):
                        nbch = min(eb, nchunks - c0)
                        base = combo_base[k] + c0
                        idx_s = esrc_sb[:, base * 8:(base + nbch) * 8]
                        idx_d = edst_sb[:, base * 8:(base + nbch) * 8]

                        def egather(tab, idxs, name):
                            t = epool.tile([P, 1, eb * P], bf16, name=name,
                                           tag=name)
                            nc.gpsimd.dma_gather(
                                out_ap=t[:, :, 0:nbch * P], in_ap=tab,
                                idxs_ap=idxs,
                                num_idxs=nbch * P, num_idxs_reg=nbch * P,
                                elem_size=HD2, transpose=True,
                                single_packet=False)
                            return t

                        gs = egather(s_g, idx_s, "gs")
                        gd = egather(d_g, idx_d, "gd")

                        ps_rt = rt_pool.tile([P, eb, 2], f32, name="ps_rt",
                                             tag="ps_rt")
                        for cc0 in range(0, nbch, 2):
                            w = min(2, nbch - cc0)
                            cols = w * P
                            ps_h = hps_pool.tile([P, 4, 2 * P], f32,
                                                 name="ps_h", tag="ps_h")
                            for q in range(4):
                                if not zb_edge:
                                    nc.tensor.matmul(
                                        ps_h[:, q, 0:cols],
                                        lhsT=brt_sb[:, q * P:(q + 1) * P],
                                        rhs=ones2_sb[:, 0:cols],
                                        start=True, stop=False)
                                nc.tensor.matmul(
                                    ps_h[:, q, 0:cols],
                                    lhsT=wsrc_sb[:, q * P:(q + 1) * P],
                                    rhs=gs[:, 0, cc0 * P:cc0 * P + cols],
                                    start=zb_edge, stop=False)
                                nc.tensor.matmul(
                                    ps_h[:, q, 0:cols],
                                    lhsT=wdst_sb[:, q * P:(q + 1) * P],
                                    rhs=gd[:, 0, cc0 * P:cc0 * P + cols],
                                    start=False, stop=True)
                            hact = jpool.tile([P, 4, 2 * P], bf16,
                                              name="hact", tag="hact")
                            nc.scalar.activation(hact[:, :, 0:cols],
                                                 ps_h[:, :, 0:cols],
                                                 AF.Lrelu, alpha=0.2)
                            for i in range(w):
                                cc = cc0 + i
                                for q in range(4):
                                    nc.tensor.matmul(
                                        ps_rt[:, cc, 0:2],
                                        lhsT=hact[:, q, i * P:(i + 1) * P],
                                        rhs=w2q_sb[:, 2 * q:2 * q + 2],
                                        start=(q == 0), stop=(q == 3))
                        # finalize batch
                        acc = fpool.tile([P, eb, 2], f32, name="acc",
                                         tag="acc")
                        nc.vector.tensor_copy(out=acc[:, 0:nbch, :],
                                              in_=ps_rt[:, 0:nbch, :])
                        tt = fpool.tile([P, eb], f32, name="tt", tag="tt")
                        nc.vector.tensor_scalar(
                            out=tt[:, 0:nbch],
                            in0=acc[:, 0:nbch, 1],
                            scalar1=br2_sb[:, 1:2], scalar2=None,
                            op0=OP.add)
                        tinv = fpool.tile([P, eb], f32, name="tinv",
                                          tag="tinv")
                        nc.vector.reciprocal(out=tinv[:, 0:nbch],
                                             in_=tt[:, 0:nbch])
                        num = fpool.tile([P, eb], f32, name="num", tag="num")
                        nc.vector.tensor_tensor(out=num[:, 0:nbch],
                                                in0=nrm_sb[:, base:base + nbch],
                                                in1=acc[:, 0:nbch, 0],
                                                op=OP.add)
                        if not zb_br2:
                            nc.vector.tensor_scalar(
                                out=num[:, 0:nbch], in0=num[:, 0:nbch],
                                scalar1=br2_sb[:, 0:1], scalar2=None,
                                op0=OP.add)
                        xx = fpool.tile([P, eb], f32, name="xx", tag="xx")
                        nc.vector.tensor_tensor(out=xx[:, 0:nbch],
                                                in0=num[:, 0:nbch],
                                                in1=tinv[:, 0:nbch],
                                                op=OP.mult)
                        osb = fpool.tile([P, eb], f32, name="osb", tag="osb")
                        nc.scalar.activation(osb[:, 0:nbch], xx[:, 0:nbch],
                                             AF.Sigmoid, scale=-1.0)
                        nc.sync.dma_start(
                            out=out_d.ap()[:, base:base + nbch],
                            in_=osb[:, 0:nbch])
    nc.finalize()
    return nc


# --------------------------------------------------------------------------
# Input staging
# --------------------------------------------------------------------------

def stage_inputs(meta, percore, host, inputs):
    npc, nb = meta["npc"], meta["nb"]
    ncores = meta["ncores"]
    old2new = host["old2new"]
    deg_pad = host["deg_pad"]
    z = np.asarray(inputs["z"], np.float32)

    zpad = np.zeros((meta["npad"], ZD), np.float32)
    zpad[old2new] = z

    def bc(v, w):
        v = np.asarray(v, np.float32).reshape(-1)
        return np.ascontiguousarray(np.broadcast_to(v, (P, w)))

    Wr1 = np.asarray(inputs["Wr1"], np.float32)
    Wt1 = np.asarray(inputs["Wt1"], np.float32)
    wsrc = np.ascontiguousarray(
        np.concatenate([Wr1[:HD2], Wt1[:HD2]], axis=1))
    wdst = np.ascontiguousarray(
        np.concatenate([Wr1[HD2:], Wt1[HD2:]], axis=1))
    # w2q: [128, 4 quarters, 2]; quarter q of the 512-wide hidden concat
    # contributes col 0 (r) for q<2 and col 1 (t) for q>=2.
    wr2 = np.asarray(inputs["Wr2"], np.float32)[:, 0]    # [HD]
    wt2 = np.asarray(inputs["Wt2"], np.float32)[:, 0]    # [HD]
    w2q = np.zeros((P, 4, 2), np.float32)
    w2q[:, 0, 0] = wr2[0:P]
    w2q[:, 1, 0] = wr2[P:HD]
    w2q[:, 2, 1] = wt2[0:P]
    w2q[:, 3, 1] = wt2[P:HD]
    brt = np.ascontiguousarray(np.concatenate(
        [np.asarray(inputs["br1"], np.float32),
         np.asarray(inputs["bt1"], np.float32)])[None, :])
    br2v = np.array([[float(np.asarray(inputs["br2"]).reshape(-1)[0]),
                      float(np.asarray(inputs["bt2"]).reshape(-1)[0])]],
                    np.float32)
    iota = np.ascontiguousarray(
        np.broadcast_to(np.arange(P, dtype=np.float32)[None, :], (P, P)))

    in_maps = []
    for c in range(ncores):
        pc = percore[c]
        degc = deg_pad[c * npc:(c + 1) * npc].reshape(nb, P)
        m = {
            "z_shard": np.ascontiguousarray(zpad[c * npc:(c + 1) * npc]),
            "deg_t": np.ascontiguousarray(degc.T),
            "gidx_lo": pc["gidx_lo"], "gidx_hi": pc["gidx_hi"],
            "dcol_lo": pc["dcol_lo"], "dcol_hi": pc["dcol_hi"],
            "dval_lo": pc["dval_lo"], "dval_hi": pc["dval_hi"],
            "eidx_src": pc["eidx_src"], "eidx_dst": pc["eidx_dst"],
            "W0": np.asarray(inputs["W0"], np.float32),
            "W1": np.asarray(inputs["W1"], np.float32),
            "W2": np.asarray(inputs["W2"], np.float32),
            "W3": np.asarray(inputs["W3"], np.float32),
            "b0c": bc(inputs["b0"], HD), "b1c": bc(inputs["b1"], HD),
            "b2c": bc(inputs["b2"], HD), "b3c": bc(inputs["b3"], HD2),
            "wsrc_cat": wsrc, "wdst_cat": wdst,
            "w2q": np.ascontiguousarray(w2q.reshape(P, 8)),
            "brt_cat": brt,
            "br2bt2": np.ascontiguousarray(np.broadcast_to(br2v, (P, 2))),
            "iota_f": iota,
            "ident_f": np.eye(P, dtype=np.float32),
        }
        in_maps.append(m)
    return in_maps


def assemble_output(meta, host, results, E):
    out = np.zeros(E, np.float32)
    slotmap = host["slotmap"]
    for c in range(meta["ncores"]):
        buf = np.asarray(results[c]["out"]).astype(np.float32)  # [P, nck]
        vals = buf.T.reshape(-1)                   # slot = chunk*P + p
        sm = slotmap[c]
        ok = sm >= 0
        out[sm[ok]] = vals[ok]
    return out


# --------------------------------------------------------------------------
# Entry point
# --------------------------------------------------------------------------

_CACHE = {}


def kernel(**inputs):
    edge_index = np.asarray(inputs["edge_index"])
    N = np.asarray(inputs["z"]).shape[0]
    E = edge_index.shape[1]

    meta, percore, host = build_plan(edge_index, N)
    zb_gcn = all(
        not np.any(np.asarray(inputs[k])) for k in ("b0", "b1", "b2", "b3"))
    zb_edge = not (np.any(np.asarray(inputs["br1"]))
                   or np.any(np.asarray(inputs["bt1"])))
    zb_br2 = not np.any(np.asarray(inputs["br2"]))
    meta["zbias"] = (zb_gcn, zb_edge, zb_br2)
    key = tuple(sorted((k, v) for k, v in meta.items()))
    if key not in _CACHE:
        _CACHE[key] = build_nc(meta, debug=False)
    nc = _CACHE[key]

    in_maps = stage_inputs(meta, percore, host, inputs)
    from concourse.bass_utils import run_bass_kernel_spmd
    import os
    trace = bool(int(os.environ.get("KERNEL_TRACE", "0")))
    res = run_bass_kernel_spmd(nc, in_maps,
                               core_ids=list(range(meta["ncores"])),
                               trace=trace)
    kernel._last_res = res
    return assemble_output(meta, host, res.results, E)

